# revision 13
# baseline (speedup 1.0000x reference)
"""Trainium2 Bass kernel for nn_MultiHeadAttention_72799695667250.

Reference computation (B=4, L=1024, D=1024, H=16, d=64, K=3):
  k/v/q = gelu(causal-circular-conv1d(x, w, b))          3 convs, D->D, K=3
  reshape to (B*H, 1024, 64)  (row-major: interleaves seq & heads)
  S = q @ k^T * 0.5 ; c = max(S, 0.1) ; attn = log_softmax(log(c))
      = log(c / sum(c))                                   (log/exp cancel)
  context = attn @ v ; out = LN(residual + context @ wf^T + bf)
  returns (out, attn)

Sharding: data-parallel over (batch b, seq-half). Core c handles b=c//2,
l in [512*half, 512*half+512). The head-interleaved reshape maps this l-range
exactly onto 8 of the 64 pseudo-batches, so attention needs no cross-core data.

Per-core math (validated in numpy): with S unscaled, c' = max(S, 0.2),
s' = rowsum(c'): attn = log(c'/s') exactly (the 0.5 cancels), and
context^T = V^T @ log(max(S^T,0.2)) - Vcolsum x log(s') (rank-1 correction).

Precision: convs for q/k and the S matmuls run in fp32r (~12-bit mantissa,
1 cycle/row); v-conv + context path run bf16. Measured end-to-end error
vs fp32 reference: attn ~7e-4, output ~6e-3 (rel to absmax).
"""

import numpy as np
import ml_dtypes

import concourse.mybir as mybir
import concourse.tile as tile
from concourse import bacc
from concourse.bass_utils import run_bass_kernel_spmd

F32 = mybir.dt.float32
F32R = mybir.dt.float32r
BF16 = mybir.dt.bfloat16
AF = mybir.ActivationFunctionType
OP = mybir.AluOpType

B, L, D = 4, 1024, 1024
H, HD = 16, 64          # heads, head dim
KW = 3                  # conv kernel width
LC = 512                # seq chunk per core
NG = 8                  # pseudo-batch groups per core
PAD_K = False           # pad attention contraction 64 -> 128 with zeros


def build_core_program():
    nc = bacc.Bacc()

    # ---- DRAM I/O ----
    xq = nc.dram_tensor("xq", [D, LC + 2], F32R, kind="ExternalInput")
    xk = nc.dram_tensor("xk", [D, LC + 2], F32R, kind="ExternalInput")
    xv = nc.dram_tensor("xv", [D, LC + 2], BF16, kind="ExternalInput")
    wq = nc.dram_tensor("wq", [KW, D, D], F32R, kind="ExternalInput")
    wk = nc.dram_tensor("wk", [KW, D, D], F32R, kind="ExternalInput")
    wv = nc.dram_tensor("wv", [KW, D, D], BF16, kind="ExternalInput")
    wf = nc.dram_tensor("wf", [D, D], BF16, kind="ExternalInput")
    res = nc.dram_tensor("res", [LC, D], F32, kind="ExternalInput")  # query + bf
    bqc = nc.dram_tensor("bqc", [128, 8], F32, kind="ExternalInput")  # bq[ot*128+p]
    bkc = nc.dram_tensor("bkc", [128, 8], F32, kind="ExternalInput")
    bvr = nc.dram_tensor("bvr", [1, D], BF16, kind="ExternalInput")
    lng = nc.dram_tensor("lng", [1, D], F32, kind="ExternalInput")
    lnb = nc.dram_tensor("lnb", [1, D], F32, kind="ExternalInput")

    attn_out = nc.dram_tensor("attn_out", [NG, 1024, 1024], F32,
                              kind="ExternalOutput")
    out = nc.dram_tensor("out", [LC, D], F32, kind="ExternalOutput")

    with tile.TileContext(nc) as tc:
        with (
            tc.tile_pool(name="singles", bufs=1) as singles,
            tc.tile_pool(name="y", bufs=1) as ypool,
        ):
            # ---- constants ----
            ones_bf = singles.tile([128, 1], BF16)
            nc.vector.memset(ones_bf, 1.0)
            eps_sb = singles.tile([128, 1], F32)
            nc.vector.memset(eps_sb, 1e-5)
            negp2 = singles.tile([128, 1], F32)
            nc.vector.memset(negp2, -0.2)
            posp2 = singles.tile([128, 1], F32)
            nc.vector.memset(posp2, 0.2)
            g_row = singles.tile([1, D], F32)
            b_row = singles.tile([1, D], F32)
            nc.sync.dma_start(g_row, lng[:, :])
            nc.sync.dma_start(b_row, lnb[:, :])
            bq_sb = singles.tile([128, 8], F32)
            bk_sb = singles.tile([128, 8], F32)
            bv_sb = singles.tile([1, D], BF16)
            ones_row = singles.tile([1, 128], BF16)
            nc.sync.dma_start(bq_sb, bqc[:, :])
            nc.sync.dma_start(bk_sb, bkc[:, :])
            nc.sync.dma_start(bv_sb, bvr[:, :])
            nc.vector.memset(ones_row, 1.0)
            g_rep = singles.tile([128, D], F32)
            b_rep = singles.tile([128, D], F32)
            nc.gpsimd.partition_broadcast(g_rep[:], g_row[:])
            nc.gpsimd.partition_broadcast(b_rep[:], b_row[:])

            # ---- conv outputs (kept in SBUF, single wide tiles) ----
            # yqt/ykt: [dd, l] transposed conv out: [128, (t, l)] = [128, 8*512]
            yqt = ypool.tile([128, 8, LC], F32R, name="yqt")
            ykt = ypool.tile([128, 8, LC], F32R, name="ykt")
            # yv: [l, dd] conv out: [128, (t, dd)] = [128, 4*1024]
            yv = ypool.tile([128, 4, D], BF16, name="yv")
            # ctx_hd: [dd, l] context: [128, (c, l)] = [128, 8*512]
            ctx_hd = ypool.tile([128, 8, LC], BF16, name="ctx_hd")

            # ================= Phase C: the three convolutions =================
            with (
                tc.tile_pool(name="xt", bufs=1) as xtp,
                tc.tile_pool(name="wstream", bufs=3) as wsp,
                tc.tile_pool(name="ps_conv", bufs=1, space="PSUM") as psc,
            ):
                xq_sb = [xtp.tile([128, LC + 2], F32R, tag=f"xq{i}", name=f"xq{i}")
                         for i in range(8)]
                xk_sb = [xtp.tile([128, LC + 2], F32R, tag=f"xk{i}", name=f"xk{i}")
                         for i in range(8)]
                xv_sb = [xtp.tile([128, LC + 2], BF16, tag=f"xv{i}", name=f"xv{i}")
                         for i in range(8)]
                for i in range(8):
                    sl = slice(128 * i, 128 * i + 128)
                    nc.sync.dma_start(xq_sb[i], xq[sl, :])
                    nc.sync.dma_start(xk_sb[i], xk[sl, :])
                    nc.sync.dma_start(xv_sb[i], xv[sl, :])

                # q/k convs: out[dd_tile, l] += W_k[i, dd].T @ X[i, l+k]
                for name, wdram, ysb, xsb, bias_sb in (
                    ("q", wq, yqt, xq_sb, bq_sb), ("k", wk, ykt, xk_sb, bk_sb),
                ):
                    psum = [psc.tile([128, LC], F32, tag=f"pc{ot}", name=f"pc{ot}")
                            for ot in range(8)]
                    for step, (k3, ic) in enumerate(
                            [(a, b_) for a in range(KW) for b_ in range(8)]):
                        wslab = wsp.tile([128, D], F32R, tag="wslab")
                        nc.sync.dma_start(
                            wslab, wdram[k3, 128 * ic:128 * ic + 128, :])
                        for ot in range(8):
                            nc.tensor.matmul(
                                psum[ot][:],
                                wslab[:, 128 * ot:128 * ot + 128],
                                xsb[ic][:, k3:k3 + LC],
                                start=(step == 0), stop=(step == KW * 8 - 1),
                            )
                    for ot in range(8):
                        nc.scalar.activation(out=ysb[:, ot, :], in_=psum[ot][:],
                                             func=AF.Gelu,
                                             bias=bias_sb[:, ot:ot + 1])

                # v conv: out[l_tile, dd] += X[i, l+k].T @ W_k[i, dd]
                psum = [psc.tile([128, LC], F32, tag=f"pc{ot}", name=f"pc{ot}")
                        for ot in range(8)]
                for step, (k3, ic) in enumerate(
                        [(a, b_) for a in range(KW) for b_ in range(8)]):
                    wslab = wsp.tile([128, D], BF16, tag="wslab")
                    nc.sync.dma_start(
                        wslab, wv[k3, 128 * ic:128 * ic + 128, :])
                    for lt in range(4):
                        for nh in range(2):
                            nc.tensor.matmul(
                                psum[lt * 2 + nh][:],
                                xv_sb[ic][:, k3 + 128 * lt:k3 + 128 * lt + 128],
                                wslab[:, 512 * nh:512 * nh + 512],
                                start=(step == 0), stop=False,
                            )
                for lt in range(4):
                    for nh in range(2):
                        nc.tensor.matmul(
                            psum[lt * 2 + nh][:], ones_row[0:1, :],
                            bv_sb[0:1, 512 * nh:512 * nh + 512],
                            start=False, stop=True)
                for lt in range(4):
                    for nh in range(2):
                        nc.scalar.activation(
                            out=yv[:, lt, 512 * nh:512 * nh + 512],
                            in_=psum[lt * 2 + nh][:], func=AF.Gelu)

            # ================= Phase A: attention per group =================
            KP = 64  # contraction rows per S matmul (row-group packed x2)
            with (
                tc.tile_pool(name="qk", bufs=2) as qkp,
                tc.tile_pool(name="vv", bufs=2) as vvp,
                tc.tile_pool(name="cc", bufs=3) as ccp,
                tc.tile_pool(name="aa", bufs=2) as aap,
                tc.tile_pool(name="lc", bufs=10) as lcp,
                tc.tile_pool(name="sm", bufs=2) as smp,
                tc.tile_pool(name="ps_s", bufs=3, space="PSUM") as pss,
                tc.tile_pool(name="ps_x", bufs=1, space="PSUM") as psx,
            ):
                for g in range(NG):
                    l0 = 64 * g
                    # -- gather Q'^T, K'^T [j, r'] (r' = h*64+l), fp32r --
                    qg = qkp.tile([128, 1024], F32R, tag="qg")
                    kg = qkp.tile([128, 1024], F32R, tag="kg")
                    qr = qkp.tile([128, 1024], F32R, tag="qr")
                    # h-even from source rows 0:64, h-odd from rows 64:128;
                    # dest cols r' = h*64 + l with h = 2t (+1)
                    for par in range(2):
                        for dup in range(2):
                            qv = qg[64 * dup:64 * dup + 64, :].rearrange(
                                "p (t l) -> p t l", t=16)
                            kv = kg[64 * dup:64 * dup + 64, :].rearrange(
                                "p (t l) -> p t l", t=16)
                            nc.sync.dma_start(
                                qv[:, par::2, :],
                                yqt[64 * par:64 * par + 64, :, l0:l0 + 64])
                            nc.sync.dma_start(
                                kv[:, par::2, :],
                                ykt[64 * par:64 * par + 64, :, l0:l0 + 64])
                    # Q in r-order (r = l*16+h) via shuffled-write DVE copy
                    nc.vector.tensor_copy(
                        qr[:, :].rearrange("p (l h) -> p h l", h=16),
                        qg[:, :])
                    kr = qkp.tile([128, 1024], F32R, tag="kr")
                    nc.vector.tensor_copy(
                        kr[:, :].rearrange("p (l h) -> p h l", h=16),
                        kg[:, :])
                    # bf16 copies for the S^T pass (feeds bf16 ctx path only)
                    qb = qkp.tile([128, 1024], BF16, tag="qb")
                    kb = qkp.tile([128, 1024], BF16, tag="kb")
                    nc.vector.tensor_copy(qb[:, :], qg[:, :].bitcast(F32))
                    nc.vector.tensor_copy(kb[:, :], kg[:, :].bitcast(F32))

                    # -- gather V' [r', j] bf16: [128, (c, j)] = [128, 512] --
                    vg = vvp.tile([128, 8, HD], BF16, tag="vg", name="vg")
                    yv_g = yv[:, g // 2, :].rearrange("p (h j) -> p h j", h=16)
                    for par in range(2):
                        nc.sync.dma_start(
                            vg[64 * par:64 * par + 64, :, :],
                            yv_g[(g % 2) * 64:(g % 2) * 64 + 64, par::2, :])

                    # -- Vcolsum [64, 1] --
                    vcs_ps = psx.tile([64, 1], F32, tag="vcs")
                    for c in range(8):
                        nc.tensor.matmul(vcs_ps[:], vg[:, c, :], ones_bf[:],
                                         start=(c == 0), stop=(c == 7))
                    vcs = smp.tile([64, 1], F32, tag="vcs_sb")
                    nc.vector.tensor_copy(vcs[:], vcs_ps[:])

                    # -- PASS 1: S rows in r-order; attn out + row sums --
                    s_cols = smp.tile([128, 8], F32, tag="s_cols")
                    for t in range(8):
                        ps = pss.tile([128, 1024], F32, tag="s")
                        for nh in range(2):
                            rg = slice(64 * nh, 64 * nh + 64)
                            nc.tensor.matmul(
                                ps[:, 512 * nh:512 * nh + 512],
                                qr[rg, 128 * t:128 * t + 128],
                                kr[rg, 512 * nh:512 * nh + 512],
                                start=True, stop=True)
                        c_sb = ccp.tile([128, 1024], F32, tag="c")
                        # clamp + rowsum accum (cols already r-ordered)
                        nc.vector.tensor_scalar(
                            out=c_sb[:], in0=ps[:], scalar1=0.2, scalar2=0.0,
                            op0=OP.max, op1=OP.add,
                            accum_out=s_cols[:, t:t + 1])
                        inv_s = smp.tile([128, 1], F32, tag="inv_s")
                        nc.vector.reciprocal(inv_s[:], s_cols[:, t:t + 1])
                        a_sb = aap.tile([128, 1024], F32, tag="a")
                        nc.scalar.activation(out=a_sb[:], in_=c_sb[:],
                                             func=AF.Ln, scale=inv_s[:])
                        nc.sync.dma_start(
                            attn_out[g, 128 * t:128 * t + 128, :], a_sb[:])

                    # -- -log(s') replicated row in r'-order [64, 1024] --
                    logs_c = smp.tile([128, 8], F32, tag="logs_c")
                    nc.scalar.activation(out=logs_c[:], in_=s_cols[:],
                                         func=AF.Ln)
                    nc.vector.tensor_scalar_mul(logs_c[:], logs_c[:], -1.0)
                    row_r = smp.tile([1, 1024], F32, tag="row_r")
                    for t in range(8):
                        nc.gpsimd.dma_start(
                            row_r[0:1, 128 * t:128 * t + 128]
                            .rearrange("o p -> o p"),
                            logs_c[:, t:t + 1])
                    row_rp = smp.tile([1, 1024], F32, tag="row_rp")
                    # r -> r' shuffle: out[h*64+l] = in[l*16+h]
                    nc.vector.tensor_copy(
                        row_rp[0:1, :].rearrange("o (h l) -> o l h", h=16),
                        row_r[0:1, :])
                    logs_rep = smp.tile([64, 1024], F32, tag="logs_rep")
                    nc.gpsimd.partition_broadcast(logs_rep[:], row_rp[:])

                    # -- PASS 2: S^T in r'-order -> log(max(S^T, 0.2)) bf16 --
                    logct = [lcp.tile([128, 1024], BF16, tag="logct", name="logct")
                             for _ in range(8)]
                    for t in range(8):
                        ps = pss.tile([128, 1024], F32, tag="s")
                        for nh in range(2):
                            rg = slice(64 * nh, 64 * nh + 64)
                            nc.tensor.matmul(
                                ps[:, 512 * nh:512 * nh + 512],
                                kb[rg, 128 * t:128 * t + 128],
                                qb[rg, 512 * nh:512 * nh + 512],
                                start=True, stop=True)
                        ct = ccp.tile([128, 1024], F32, tag="ct")
                        if t % 2 == 0:
                            nc.vector.tensor_scalar_max(ct[:], ps[:], 0.2)
                            nc.scalar.activation(out=logct[t][:], in_=ct[:],
                                                 func=AF.Ln)
                        else:
                            nc.scalar.activation(out=ct[:], in_=ps[:],
                                                 func=AF.Relu, bias=negp2[:])
                            nc.scalar.activation(out=logct[t][:], in_=ct[:],
                                                 func=AF.Ln, bias=posp2[:])

                    # -- PASS 3: context^T = V'^T @ logct - Vcs x log(s') --
                    ctxT = aap.tile([64, 1024], BF16, tag="ctxT")
                    for nh in range(2):
                        px = psx.tile([64, 512], F32, tag="x")
                        for c in range(8):
                            nc.tensor.matmul(
                                px[:], vg[:, c, :],
                                logct[c][:, 512 * nh:512 * nh + 512],
                                start=(c == 0), stop=(c == 7))
                        nc.vector.scalar_tensor_tensor(
                            out=ctxT[:, 512 * nh:512 * nh + 512],
                            in0=logs_rep[:, 512 * nh:512 * nh + 512],
                            scalar=vcs[:], in1=px[:],
                            op0=OP.mult, op1=OP.add)

                    # -- scatter context into ctx_hd [dd, l]: 2 DMAs --
                    ctxT_v = ctxT[:, :].rearrange("p (h l) -> p h l", h=16)
                    for par in range(2):
                        nc.sync.dma_start(
                            ctx_hd[64 * par:64 * par + 64, :, l0:l0 + 64],
                            ctxT_v[:, par::2, :])

            # ================= Phase P: projection + layernorm =================
            with (
                tc.tile_pool(name="wf", bufs=1) as wfp,
                tc.tile_pool(name="pp", bufs=2) as ppp,
                tc.tile_pool(name="ps_p", bufs=2, space="PSUM") as psp,
            ):
                wf_sb = [wfp.tile([128, D], BF16, tag=f"wf{c}", name=f"wfsb{c}")
                         for c in range(8)]
                for c in range(8):
                    nc.sync.dma_start(wf_sb[c],
                                      wf[128 * c:128 * c + 128, :])
                for lt in range(4):
                    ps = psp.tile([128, 1024], F32, tag="p")
                    for nh in range(2):
                        for c in range(8):
                            nc.tensor.matmul(
                                ps[:, 512 * nh:512 * nh + 512],
                                ctx_hd[:, c, 128 * lt:128 * lt + 128],
                                wf_sb[c][:, 512 * nh:512 * nh + 512],
                                start=(c == 0), stop=(c == 7))
                    res_sb = ppp.tile([128, D], F32, tag="res")
                    nc.sync.dma_start(res_sb,
                                      res[128 * lt:128 * lt + 128, :])
                    x_sb = ppp.tile([128, D], F32, tag="x")
                    nc.vector.tensor_tensor(x_sb[:], ps[:], res_sb[:], OP.add)
                    # layernorm stats (1024 = 2 x 512 bn_stats subgroups)
                    stats = ppp.tile([128, 2, 6], F32, tag="stats")
                    xg = x_sb[:].rearrange("p (s d) -> p s d", s=2)
                    for sg in range(2):
                        nc.vector.bn_stats(out=stats[:, sg, :],
                                           in_=xg[:, sg, :])
                    mv = ppp.tile([128, 2], F32, tag="mv")
                    nc.vector.bn_aggr(out=mv[:], in_=stats[:])
                    std = ppp.tile([128, 1], F32, tag="std")
                    nc.scalar.activation(out=std[:], in_=mv[:, 1:2],
                                         func=AF.Sqrt, bias=eps_sb[:])
                    istd = ppp.tile([128, 1], F32, tag="istd")
                    nc.vector.reciprocal(istd[:], std[:])
                    y_sb = ppp.tile([128, D], F32, tag="y")
                    nc.vector.tensor_scalar(
                        out=y_sb[:], in0=x_sb[:],
                        scalar1=mv[:, 0:1], scalar2=istd[:],
                        op0=OP.subtract, op1=OP.mult)
                    z_sb = ppp.tile([128, D], F32, tag="z")
                    nc.vector.tensor_tensor(z_sb[:], y_sb[:], g_rep[:],
                                            OP.mult)
                    o_sb = ppp.tile([128, D], F32, tag="o")
                    nc.vector.tensor_tensor(o_sb[:], z_sb[:], b_rep[:],
                                            OP.add)
                    nc.sync.dma_start(out[128 * lt:128 * lt + 128, :],
                                      o_sb[:])

    nc.compile()
    return nc


_CACHED_NC = None
LAST_RESULTS = None


def _get_nc():
    global _CACHED_NC
    if _CACHED_NC is None:
        _CACHED_NC = build_core_program()
    return _CACHED_NC


def kernel(key, value, query, wk, bk, wv, bv, wq, bq, wf, bf, ln_g, ln_b):
    key = np.asarray(key, np.float32)
    value = np.asarray(value, np.float32)
    query = np.asarray(query, np.float32)
    f32 = lambda x: np.asarray(x, np.float32)
    wk, bk, wv, bv, wq, bq = map(f32, (wk, bk, wv, bv, wq, bq))
    wf, bf, ln_g, ln_b = map(f32, (wf, bf, ln_g, ln_b))

    # Weights pre-transposed host-side to [k, i, dd] ([dd, o] for wf). Conv
    # biases ride along: bq/bk as per-partition gelu bias columns, bv as a
    # rank-1 matmul row, bf folded into the residual.
    wq_t = np.ascontiguousarray(wq.transpose(2, 1, 0))
    wk_t = np.ascontiguousarray(wk.transpose(2, 1, 0))
    wv_t = np.ascontiguousarray(wv.transpose(2, 1, 0)).astype(ml_dtypes.bfloat16)
    wf_t = np.ascontiguousarray(wf.T).astype(ml_dtypes.bfloat16)

    nc = _get_nc()

    in_maps = []
    for core in range(8):
        b, half = core // 2, core % 2
        s0 = half * LC
        cols = (np.arange(s0 - 2, s0 + LC) % L)
        in_maps.append({
            "xq": np.ascontiguousarray(query[b].T[:, cols]),
            "xk": np.ascontiguousarray(key[b].T[:, cols]),
            "xv": np.ascontiguousarray(value[b].T[:, cols]).astype(
                ml_dtypes.bfloat16),
            "wq": wq_t, "wk": wk_t, "wv": wv_t, "wf": wf_t,
            "bqc": np.ascontiguousarray(bq.reshape(8, 128).T),
            "bkc": np.ascontiguousarray(bk.reshape(8, 128).T),
            "bvr": np.ascontiguousarray(bv[None, :]).astype(ml_dtypes.bfloat16),
            "res": np.ascontiguousarray(query[b, s0:s0 + LC] + bf),
            "lng": np.ascontiguousarray(ln_g[None, :]),
            "lnb": np.ascontiguousarray(ln_b[None, :]),
        })

    import os
    trace = bool(int(os.environ.get("KERNEL_TRACE", "0")))
    results = run_bass_kernel_spmd(nc, in_maps, core_ids=list(range(8)),
                                   trace=trace)
    global LAST_RESULTS
    LAST_RESULTS = results

    output = np.zeros((B, L, D), np.float32)
    attn = np.zeros((B * H, L, L), np.float32)
    for core in range(8):
        b, half = core // 2, core % 2
        s0 = half * LC
        r = results.results[core]
        output[b, s0:s0 + LC] = r["out"]
        attn[b * H + half * NG:b * H + half * NG + NG] = r["attn_out"]
    return (output, attn)


if __name__ == "__main__":
    # quick self-build check
    nc = build_core_program()
    print("build + compile OK")


# revision 15
# speedup vs baseline: 1.1059x; 1.1059x over previous
"""Trainium2 Bass kernel for nn_MultiHeadAttention_72799695667250.

Reference computation (B=4, L=1024, D=1024, H=16, d=64, K=3):
  k/v/q = gelu(causal-circular-conv1d(x, w, b))          3 convs, D->D, K=3
  reshape to (B*H, 1024, 64)  (row-major: interleaves seq & heads)
  S = q @ k^T * 0.5 ; c = max(S, 0.1) ; attn = log_softmax(log(c))
      = log(c / sum(c))                                   (log/exp cancel)
  context = attn @ v ; out = LN(residual + context @ wf^T + bf)
  returns (out, attn)

Sharding: data-parallel over (batch b, seq-half). Core c handles b=c//2,
l in [512*half, 512*half+512). The head-interleaved reshape maps this l-range
exactly onto 8 of the 64 pseudo-batches, so attention needs no cross-core data.

Per-core math (validated in numpy): with S unscaled, c' = max(S, 0.2),
s' = rowsum(c'): attn = log(c'/s') exactly (the 0.5 cancels), and
context^T = V^T @ log(max(S^T,0.2)) - Vcolsum x log(s') (rank-1 correction).

Precision: convs for q/k and the S matmuls run in fp32r (~12-bit mantissa,
1 cycle/row); v-conv + context path run bf16. Measured end-to-end error
vs fp32 reference: attn ~7e-4, output ~6e-3 (rel to absmax).
"""

import numpy as np
import ml_dtypes

import concourse.mybir as mybir
import concourse.tile as tile
from concourse import bacc
from concourse.bass_utils import run_bass_kernel_spmd

F32 = mybir.dt.float32
F32R = mybir.dt.float32r
BF16 = mybir.dt.bfloat16
AF = mybir.ActivationFunctionType
OP = mybir.AluOpType

B, L, D = 4, 1024, 1024
H, HD = 16, 64          # heads, head dim
KW = 3                  # conv kernel width
LC = 512                # seq chunk per core
NG = 8                  # pseudo-batch groups per core
PAD_K = False           # pad attention contraction 64 -> 128 with zeros


def build_core_program():
    nc = bacc.Bacc()

    # ---- DRAM I/O ----
    xq = nc.dram_tensor("xq", [D, LC + 2], F32R, kind="ExternalInput")
    xk = nc.dram_tensor("xk", [D, LC + 2], F32R, kind="ExternalInput")
    xv = nc.dram_tensor("xv", [D, LC + 2], BF16, kind="ExternalInput")
    wq = nc.dram_tensor("wq", [KW, D, D], F32R, kind="ExternalInput")
    wk = nc.dram_tensor("wk", [KW, D, D], F32R, kind="ExternalInput")
    wv = nc.dram_tensor("wv", [KW, D, D], BF16, kind="ExternalInput")
    wf = nc.dram_tensor("wf", [D, D], BF16, kind="ExternalInput")
    res = nc.dram_tensor("res", [LC, D], F32, kind="ExternalInput")  # query + bf
    bqc = nc.dram_tensor("bqc", [128, 8], F32, kind="ExternalInput")  # bq[ot*128+p]
    bkc = nc.dram_tensor("bkc", [128, 8], F32, kind="ExternalInput")
    bvr = nc.dram_tensor("bvr", [1, D], BF16, kind="ExternalInput")
    lng = nc.dram_tensor("lng", [1, D], F32, kind="ExternalInput")
    lnb = nc.dram_tensor("lnb", [1, D], F32, kind="ExternalInput")

    attn_out = nc.dram_tensor("attn_out", [NG, 1024, 1024], F32,
                              kind="ExternalOutput")
    out = nc.dram_tensor("out", [LC, D], F32, kind="ExternalOutput")

    with tile.TileContext(nc) as tc:
        with (
            tc.tile_pool(name="singles", bufs=1) as singles,
            tc.tile_pool(name="y", bufs=1) as ypool,
        ):
            # ---- constants ----
            ones_bf = singles.tile([128, 1], BF16)
            nc.vector.memset(ones_bf, 1.0)
            eps_sb = singles.tile([128, 1], F32)
            nc.vector.memset(eps_sb, 1e-5)
            negp2 = singles.tile([128, 1], F32)
            nc.vector.memset(negp2, -0.2)
            posp2 = singles.tile([128, 1], F32)
            nc.vector.memset(posp2, 0.2)
            g_row = singles.tile([1, D], F32)
            b_row = singles.tile([1, D], F32)
            nc.sync.dma_start(g_row, lng[:, :])
            nc.sync.dma_start(b_row, lnb[:, :])
            bq_sb = singles.tile([128, 8], F32)
            bk_sb = singles.tile([128, 8], F32)
            bv_sb = singles.tile([1, D], BF16)
            ones_row = singles.tile([1, 128], BF16)
            identity_f = singles.tile([128, 128], F32)
            nc.sync.dma_start(bq_sb, bqc[:, :])
            nc.sync.dma_start(bk_sb, bkc[:, :])
            nc.sync.dma_start(bv_sb, bvr[:, :])
            nc.vector.memset(ones_row, 1.0)
            from concourse.masks import make_identity
            make_identity(nc, identity_f)
            g_rep = singles.tile([128, D], F32)
            b_rep = singles.tile([128, D], F32)
            nc.gpsimd.partition_broadcast(g_rep[:], g_row[:])
            nc.gpsimd.partition_broadcast(b_rep[:], b_row[:])

            # ---- conv outputs (kept in SBUF, single wide tiles) ----
            # yqt/ykt: [dd, l] transposed conv out: [128, (t, l)] = [128, 8*512]
            yqt = ypool.tile([128, 8, LC], F32R, name="yqt")
            ykt = ypool.tile([128, 8, LC], F32R, name="ykt")
            # yv: [l, dd] conv out: [128, (t, dd)] = [128, 4*1024]
            yv = ypool.tile([128, 4, D], BF16, name="yv")
            # ctx_hd: [dd, l] context: [128, (c, l)] = [128, 8*512]
            ctx_hd = ypool.tile([128, 8, LC], BF16, name="ctx_hd")

            # ================= Phase C: the three convolutions =================
            with (
                tc.tile_pool(name="xt", bufs=1) as xtp,
                tc.tile_pool(name="wstream", bufs=3) as wsp,
                tc.tile_pool(name="ps_conv", bufs=1, space="PSUM") as psc,
            ):
                xq_sb = [xtp.tile([128, LC + 2], F32R, tag=f"xq{i}", name=f"xq{i}")
                         for i in range(8)]
                xk_sb = [xtp.tile([128, LC + 2], F32R, tag=f"xk{i}", name=f"xk{i}")
                         for i in range(8)]
                xv_sb = [xtp.tile([128, LC + 2], BF16, tag=f"xv{i}", name=f"xv{i}")
                         for i in range(8)]
                for i in range(8):
                    sl = slice(128 * i, 128 * i + 128)
                    nc.sync.dma_start(xq_sb[i], xq[sl, :])
                    nc.sync.dma_start(xk_sb[i], xk[sl, :])
                    nc.sync.dma_start(xv_sb[i], xv[sl, :])

                # q/k convs: out[dd_tile, l] += W_k[i, dd].T @ X[i, l+k]
                for name, wdram, ysb, xsb, bias_sb in (
                    ("q", wq, yqt, xq_sb, bq_sb), ("k", wk, ykt, xk_sb, bk_sb),
                ):
                    psum = [psc.tile([128, LC], F32, tag=f"pc{ot}", name=f"pc{ot}")
                            for ot in range(8)]
                    for step, (k3, ic) in enumerate(
                            [(a, b_) for a in range(KW) for b_ in range(8)]):
                        wslab = wsp.tile([128, D], F32R, tag="wslab")
                        nc.sync.dma_start(
                            wslab, wdram[k3, 128 * ic:128 * ic + 128, :])
                        for ot in range(8):
                            nc.tensor.matmul(
                                psum[ot][:],
                                wslab[:, 128 * ot:128 * ot + 128],
                                xsb[ic][:, k3:k3 + LC],
                                start=(step == 0), stop=(step == KW * 8 - 1),
                            )
                    for ot in range(8):
                        nc.scalar.activation(out=ysb[:, ot, :], in_=psum[ot][:],
                                             func=AF.Gelu,
                                             bias=bias_sb[:, ot:ot + 1])

                # v conv: out[l_tile, dd] += X[i, l+k].T @ W_k[i, dd]
                psum = [psc.tile([128, LC], F32, tag=f"pc{ot}", name=f"pc{ot}")
                        for ot in range(8)]
                for step, (k3, ic) in enumerate(
                        [(a, b_) for a in range(KW) for b_ in range(8)]):
                    wslab = wsp.tile([128, D], BF16, tag="wslab")
                    nc.sync.dma_start(
                        wslab, wv[k3, 128 * ic:128 * ic + 128, :])
                    for lt in range(4):
                        for nh in range(2):
                            nc.tensor.matmul(
                                psum[lt * 2 + nh][:],
                                xv_sb[ic][:, k3 + 128 * lt:k3 + 128 * lt + 128],
                                wslab[:, 512 * nh:512 * nh + 512],
                                start=(step == 0), stop=False,
                            )
                for lt in range(4):
                    for nh in range(2):
                        nc.tensor.matmul(
                            psum[lt * 2 + nh][:], ones_row[0:1, :],
                            bv_sb[0:1, 512 * nh:512 * nh + 512],
                            start=False, stop=True)
                for lt in range(4):
                    for nh in range(2):
                        nc.scalar.activation(
                            out=yv[:, lt, 512 * nh:512 * nh + 512],
                            in_=psum[lt * 2 + nh][:], func=AF.Gelu)

            # ================= Phase A: attention per group =================
            KP = 64  # contraction rows per S matmul (row-group packed x2)
            with (
                tc.tile_pool(name="qk", bufs=2) as qkp,
                tc.tile_pool(name="vv", bufs=2) as vvp,
                tc.tile_pool(name="cc", bufs=3) as ccp,
                tc.tile_pool(name="aa", bufs=2) as aap,
                tc.tile_pool(name="lc", bufs=10) as lcp,
                tc.tile_pool(name="sm", bufs=3) as smp,
                tc.tile_pool(name="ps_s", bufs=2, space="PSUM") as pss,
                tc.tile_pool(name="ps_x", bufs=1, space="PSUM") as psx,
            ):
                for g in range(NG):
                    l0 = 64 * g
                    # -- gather Q'^T, K'^T [j, r'] (r' = h*64+l), fp32r --
                    qg = qkp.tile([128, 1024], F32R, tag="qg")
                    kg = qkp.tile([128, 1024], F32R, tag="kg")
                    qr = qkp.tile([128, 1024], F32R, tag="qr")
                    # h-even from source rows 0:64, h-odd from rows 64:128;
                    # dest cols r' = h*64 + l with h = 2t (+1)
                    for par in range(2):
                        for dup in range(2):
                            qv = qg[64 * dup:64 * dup + 64, :].rearrange(
                                "p (t l) -> p t l", t=16)
                            kv = kg[64 * dup:64 * dup + 64, :].rearrange(
                                "p (t l) -> p t l", t=16)
                            nc.sync.dma_start(
                                qv[:, par::2, :],
                                yqt[64 * par:64 * par + 64, :, l0:l0 + 64])
                            nc.sync.dma_start(
                                kv[:, par::2, :],
                                ykt[64 * par:64 * par + 64, :, l0:l0 + 64])
                    # Q in r-order (r = l*16+h) via shuffled-write DVE copy
                    nc.vector.tensor_copy(
                        qr[:, :].rearrange("p (l h) -> p h l", h=16),
                        qg[:, :])
                    kr = qkp.tile([128, 1024], F32R, tag="kr")
                    nc.vector.tensor_copy(
                        kr[:, :].rearrange("p (l h) -> p h l", h=16),
                        kg[:, :])
                    # bf16 copies for the S^T pass (feeds bf16 ctx path only)
                    qb = qkp.tile([128, 1024], BF16, tag="qb")
                    kb = qkp.tile([128, 1024], BF16, tag="kb")
                    nc.vector.tensor_copy(qb[:, :], qg[:, :].bitcast(F32))
                    nc.vector.tensor_copy(kb[:, :], kg[:, :].bitcast(F32))

                    # -- gather V' [r', j] bf16: [128, (c, j)] = [128, 512] --
                    vg = vvp.tile([128, 8, HD], BF16, tag="vg", name="vg")
                    yv_g = yv[:, g // 2, :].rearrange("p (h j) -> p h j", h=16)
                    for par in range(2):
                        nc.sync.dma_start(
                            vg[64 * par:64 * par + 64, :, :],
                            yv_g[(g % 2) * 64:(g % 2) * 64 + 64, par::2, :])

                    # -- Vcolsum [64, 1] (negated, for the rank-1 fold) --
                    vcs_ps = psx.tile([64, 1], F32, tag="vcs")
                    for c in range(8):
                        nc.tensor.matmul(vcs_ps[:], vg[:, c, :], ones_bf[:],
                                         start=(c == 0), stop=(c == 7))
                    vcs = smp.tile([64, 1], F32, tag="vcs_sb")
                    nc.vector.tensor_scalar_mul(vcs[:], vcs_ps[:], -1.0)

                    # -- PASS 1: S rows in r-order; attn out + row sums --
                    s_cols = smp.tile([128, 8], F32, tag="s_cols")
                    for t in range(8):
                        ps = pss.tile([128, 1024], F32, tag="s")
                        for nh in range(2):
                            rg = slice(64 * nh, 64 * nh + 64)
                            nc.tensor.matmul(
                                ps[:, 512 * nh:512 * nh + 512],
                                qr[rg, 128 * t:128 * t + 128],
                                kr[rg, 512 * nh:512 * nh + 512],
                                start=True, stop=True)
                        c_sb = ccp.tile([128, 1024], F32, tag="c")
                        # clamp + rowsum accum (cols already r-ordered)
                        nc.vector.tensor_scalar(
                            out=c_sb[:], in0=ps[:], scalar1=0.2, scalar2=0.0,
                            op0=OP.max, op1=OP.add,
                            accum_out=s_cols[:, t:t + 1])
                        inv_s = smp.tile([128, 1], F32, tag="inv_s")
                        nc.vector.reciprocal(inv_s[:], s_cols[:, t:t + 1])
                        a_sb = aap.tile([128, 1024], F32, tag="a")
                        nc.scalar.activation(out=a_sb[:], in_=c_sb[:],
                                             func=AF.Ln, scale=inv_s[:])
                        nc.sync.dma_start(
                            attn_out[g, 128 * t:128 * t + 128, :], a_sb[:])

                    # -- log(s') replicated row in r'-order [64, 1024] --
                    logs_c = smp.tile([128, 8], F32, tag="logs_c")
                    nc.scalar.activation(out=logs_c[:], in_=s_cols[:],
                                         func=AF.Ln)
                    lt_ps = psx.tile([8, 128], F32, tag="lt")
                    nc.tensor.transpose(lt_ps[:], logs_c[:], identity_f[:])
                    lt_sb = smp.tile([8, 128], F32, tag="lt_sb")
                    nc.vector.tensor_copy(lt_sb[:], lt_ps[:])
                    row_r = smp.tile([1, 1024], F32, tag="row_r")
                    nc.sync.dma_start(
                        row_r[0:1, :].rearrange("o (t p) -> o t p", t=8),
                        lt_sb[:, :])
                    row_rp = smp.tile([1, 1024], F32, tag="row_rp")
                    # r -> r' shuffle: out[h*64+l] = in[l*16+h]
                    nc.vector.tensor_copy(
                        row_rp[0:1, :].rearrange("o (h l) -> o l h", h=16),
                        row_r[0:1, :])
                    logs_rep = smp.tile([64, 1024], F32, tag="logs_rep")
                    nc.gpsimd.partition_broadcast(logs_rep[:], row_rp[:])

                    # -- PASS 2: S^T in r'-order -> log(max(S^T, 0.2)) bf16 --
                    logct = [lcp.tile([128, 1024], BF16, tag="logct", name="logct")
                             for _ in range(8)]
                    for t in range(8):
                        ps = pss.tile([128, 1024], F32, tag="s")
                        for nh in range(2):
                            rg = slice(64 * nh, 64 * nh + 64)
                            nc.tensor.matmul(
                                ps[:, 512 * nh:512 * nh + 512],
                                kb[rg, 128 * t:128 * t + 128],
                                qb[rg, 512 * nh:512 * nh + 512],
                                start=True, stop=True)
                        ct = ccp.tile([128, 1024], F32, tag="ct")
                        nc.vector.tensor_scalar_max(ct[:], ps[:], 0.2)
                        nc.scalar.activation(out=logct[t][:], in_=ct[:],
                                             func=AF.Ln)

                    # -- PASS 3: context^T = V'^T @ logct - Vcs x log(s') --
                    ctxT = aap.tile([64, 1024], BF16, tag="ctxT")
                    for nh in range(2):
                        px = psx.tile([64, 512], F32, tag="x")
                        for c in range(8):
                            nc.tensor.matmul(
                                px[:], vg[:, c, :],
                                logct[c][:, 512 * nh:512 * nh + 512],
                                start=(c == 0), stop=(c == 7))
                        nc.vector.scalar_tensor_tensor(
                            out=ctxT[:, 512 * nh:512 * nh + 512],
                            in0=logs_rep[:, 512 * nh:512 * nh + 512],
                            scalar=vcs[:], in1=px[:],
                            op0=OP.mult, op1=OP.add)

                    # -- scatter context into ctx_hd [dd, l]: 2 DMAs --
                    ctxT_v = ctxT[:, :].rearrange("p (h l) -> p h l", h=16)
                    for par in range(2):
                        nc.sync.dma_start(
                            ctx_hd[64 * par:64 * par + 64, :, l0:l0 + 64],
                            ctxT_v[:, par::2, :])

            # ================= Phase P: projection + layernorm =================
            with (
                tc.tile_pool(name="wf", bufs=1) as wfp,
                tc.tile_pool(name="pp", bufs=2) as ppp,
                tc.tile_pool(name="ps_p", bufs=2, space="PSUM") as psp,
            ):
                wf_sb = [wfp.tile([128, D], BF16, tag=f"wf{c}", name=f"wfsb{c}")
                         for c in range(8)]
                for c in range(8):
                    nc.sync.dma_start(wf_sb[c],
                                      wf[128 * c:128 * c + 128, :])
                for lt in range(4):
                    ps = psp.tile([128, 1024], F32, tag="p")
                    for nh in range(2):
                        for c in range(8):
                            nc.tensor.matmul(
                                ps[:, 512 * nh:512 * nh + 512],
                                ctx_hd[:, c, 128 * lt:128 * lt + 128],
                                wf_sb[c][:, 512 * nh:512 * nh + 512],
                                start=(c == 0), stop=(c == 7))
                    res_sb = ppp.tile([128, D], F32, tag="res")
                    nc.sync.dma_start(res_sb,
                                      res[128 * lt:128 * lt + 128, :])
                    x_sb = ppp.tile([128, D], F32, tag="x")
                    nc.vector.tensor_tensor(x_sb[:], ps[:], res_sb[:], OP.add)
                    # layernorm stats (1024 = 2 x 512 bn_stats subgroups)
                    stats = ppp.tile([128, 2, 6], F32, tag="stats")
                    xg = x_sb[:].rearrange("p (s d) -> p s d", s=2)
                    for sg in range(2):
                        nc.vector.bn_stats(out=stats[:, sg, :],
                                           in_=xg[:, sg, :])
                    mv = ppp.tile([128, 2], F32, tag="mv")
                    nc.vector.bn_aggr(out=mv[:], in_=stats[:])
                    std = ppp.tile([128, 1], F32, tag="std")
                    nc.scalar.activation(out=std[:], in_=mv[:, 1:2],
                                         func=AF.Sqrt, bias=eps_sb[:])
                    istd = ppp.tile([128, 1], F32, tag="istd")
                    nc.vector.reciprocal(istd[:], std[:])
                    y_sb = ppp.tile([128, D], F32, tag="y")
                    nc.vector.tensor_scalar(
                        out=y_sb[:], in0=x_sb[:],
                        scalar1=mv[:, 0:1], scalar2=istd[:],
                        op0=OP.subtract, op1=OP.mult)
                    z_sb = ppp.tile([128, D], F32, tag="z")
                    nc.vector.tensor_tensor(z_sb[:], y_sb[:], g_rep[:],
                                            OP.mult)
                    o_sb = ppp.tile([128, D], F32, tag="o")
                    nc.vector.tensor_tensor(o_sb[:], z_sb[:], b_rep[:],
                                            OP.add)
                    nc.sync.dma_start(out[128 * lt:128 * lt + 128, :],
                                      o_sb[:])

    nc.compile()
    return nc


_CACHED_NC = None
LAST_RESULTS = None


def _get_nc():
    global _CACHED_NC
    if _CACHED_NC is None:
        _CACHED_NC = build_core_program()
    return _CACHED_NC


def kernel(key, value, query, wk, bk, wv, bv, wq, bq, wf, bf, ln_g, ln_b):
    key = np.asarray(key, np.float32)
    value = np.asarray(value, np.float32)
    query = np.asarray(query, np.float32)
    f32 = lambda x: np.asarray(x, np.float32)
    wk, bk, wv, bv, wq, bq = map(f32, (wk, bk, wv, bv, wq, bq))
    wf, bf, ln_g, ln_b = map(f32, (wf, bf, ln_g, ln_b))

    # Weights pre-transposed host-side to [k, i, dd] ([dd, o] for wf). Conv
    # biases ride along: bq/bk as per-partition gelu bias columns, bv as a
    # rank-1 matmul row, bf folded into the residual.
    wq_t = np.ascontiguousarray(wq.transpose(2, 1, 0))
    wk_t = np.ascontiguousarray(wk.transpose(2, 1, 0))
    wv_t = np.ascontiguousarray(wv.transpose(2, 1, 0)).astype(ml_dtypes.bfloat16)
    wf_t = np.ascontiguousarray(wf.T).astype(ml_dtypes.bfloat16)

    nc = _get_nc()

    in_maps = []
    for core in range(8):
        b, half = core // 2, core % 2
        s0 = half * LC
        cols = (np.arange(s0 - 2, s0 + LC) % L)
        in_maps.append({
            "xq": np.ascontiguousarray(query[b].T[:, cols]),
            "xk": np.ascontiguousarray(key[b].T[:, cols]),
            "xv": np.ascontiguousarray(value[b].T[:, cols]).astype(
                ml_dtypes.bfloat16),
            "wq": wq_t, "wk": wk_t, "wv": wv_t, "wf": wf_t,
            "bqc": np.ascontiguousarray(bq.reshape(8, 128).T),
            "bkc": np.ascontiguousarray(bk.reshape(8, 128).T),
            "bvr": np.ascontiguousarray(bv[None, :]).astype(ml_dtypes.bfloat16),
            "res": np.ascontiguousarray(query[b, s0:s0 + LC] + bf),
            "lng": np.ascontiguousarray(ln_g[None, :]),
            "lnb": np.ascontiguousarray(ln_b[None, :]),
        })

    import os
    trace = bool(int(os.environ.get("KERNEL_TRACE", "0")))
    results = run_bass_kernel_spmd(nc, in_maps, core_ids=list(range(8)),
                                   trace=trace)
    global LAST_RESULTS
    LAST_RESULTS = results

    output = np.zeros((B, L, D), np.float32)
    attn = np.zeros((B * H, L, L), np.float32)
    for core in range(8):
        b, half = core // 2, core % 2
        s0 = half * LC
        r = results.results[core]
        output[b, s0:s0 + LC] = r["out"]
        attn[b * H + half * NG:b * H + half * NG + NG] = r["attn_out"]
    return (output, attn)


if __name__ == "__main__":
    # quick self-build check
    nc = build_core_program()
    print("build + compile OK")


# revision 17
# speedup vs baseline: 1.1833x; 1.0700x over previous
"""Trainium2 Bass kernel for nn_MultiHeadAttention_72799695667250.

Reference computation (B=4, L=1024, D=1024, H=16, d=64, K=3):
  k/v/q = gelu(causal-circular-conv1d(x, w, b))          3 convs, D->D, K=3
  reshape to (B*H, 1024, 64)  (row-major: interleaves seq & heads)
  S = q @ k^T * 0.5 ; c = max(S, 0.1) ; attn = log_softmax(log(c))
      = log(c / sum(c))                                   (log/exp cancel)
  context = attn @ v ; out = LN(residual + context @ wf^T + bf)
  returns (out, attn)

Sharding: data-parallel over (batch b, seq-half). Core c handles b=c//2,
l in [512*half, 512*half+512). The head-interleaved reshape maps this l-range
exactly onto 8 of the 64 pseudo-batches, so attention needs no cross-core data.

Per-core math (validated in numpy): with S unscaled, c' = max(S, 0.2),
s' = rowsum(c'): attn = log(c'/s') exactly (the 0.5 cancels), and
context^T = V^T @ log(max(S^T,0.2)) - Vcolsum x log(s') (rank-1 correction).

Precision: convs for q/k and the S matmuls run in fp32r (~12-bit mantissa,
1 cycle/row); v-conv + context path run bf16. Measured end-to-end error
vs fp32 reference: attn ~7e-4, output ~6e-3 (rel to absmax).
"""

import numpy as np
import ml_dtypes

import concourse.mybir as mybir
import concourse.tile as tile
from concourse import bacc
from concourse.bass_utils import run_bass_kernel_spmd

F32 = mybir.dt.float32
F32R = mybir.dt.float32r
BF16 = mybir.dt.bfloat16
AF = mybir.ActivationFunctionType
OP = mybir.AluOpType

B, L, D = 4, 1024, 1024
H, HD = 16, 64          # heads, head dim
KW = 3                  # conv kernel width
LC = 512                # seq chunk per core
NG = 8                  # pseudo-batch groups per core
PAD_K = False           # pad attention contraction 64 -> 128 with zeros


def build_core_program():
    nc = bacc.Bacc()

    # ---- DRAM I/O ----
    xq = nc.dram_tensor("xq", [D, LC + 2], F32R, kind="ExternalInput")
    xk = nc.dram_tensor("xk", [D, LC + 2], F32R, kind="ExternalInput")
    xv = nc.dram_tensor("xv", [D, LC + 2], BF16, kind="ExternalInput")
    wq = nc.dram_tensor("wq", [KW, D, D], F32R, kind="ExternalInput")
    wk = nc.dram_tensor("wk", [KW, D, D], F32R, kind="ExternalInput")
    wv = nc.dram_tensor("wv", [KW, D, D], BF16, kind="ExternalInput")
    wf = nc.dram_tensor("wf", [D, D], BF16, kind="ExternalInput")
    res = nc.dram_tensor("res", [LC, D], F32, kind="ExternalInput")  # query + bf
    bqc = nc.dram_tensor("bqc", [128, 8], F32, kind="ExternalInput")  # bq[ot*128+p]
    bkc = nc.dram_tensor("bkc", [128, 8], F32, kind="ExternalInput")
    bvr = nc.dram_tensor("bvr", [1, D], BF16, kind="ExternalInput")
    lng = nc.dram_tensor("lng", [1, D], F32, kind="ExternalInput")
    lnb = nc.dram_tensor("lnb", [1, D], F32, kind="ExternalInput")

    attn_out = nc.dram_tensor("attn_out", [NG, 1024, 1024], F32,
                              kind="ExternalOutput")
    out = nc.dram_tensor("out", [LC, D], F32, kind="ExternalOutput")

    with tile.TileContext(nc) as tc:
        with (
            tc.tile_pool(name="singles", bufs=1) as singles,
            tc.tile_pool(name="y", bufs=1) as ypool,
        ):
            # ---- constants ----
            ones_bf = singles.tile([128, 1], BF16)
            nc.vector.memset(ones_bf, 1.0)
            eps_sb = singles.tile([128, 1], F32)
            nc.vector.memset(eps_sb, 1e-5)
            negp2 = singles.tile([128, 1], F32)
            nc.vector.memset(negp2, -0.2)
            posp2 = singles.tile([128, 1], F32)
            nc.vector.memset(posp2, 0.2)
            g_row = singles.tile([1, D], F32)
            b_row = singles.tile([1, D], F32)
            nc.sync.dma_start(g_row, lng[:, :])
            nc.sync.dma_start(b_row, lnb[:, :])
            bq_sb = singles.tile([128, 8], F32)
            bk_sb = singles.tile([128, 8], F32)
            bv_sb = singles.tile([1, D], BF16)
            ones_row = singles.tile([1, 128], BF16)
            identity_f = singles.tile([128, 128], F32)
            nc.sync.dma_start(bq_sb, bqc[:, :])
            nc.sync.dma_start(bk_sb, bkc[:, :])
            nc.sync.dma_start(bv_sb, bvr[:, :])
            nc.vector.memset(ones_row, 1.0)
            from concourse.masks import make_identity
            make_identity(nc, identity_f)
            g_rep = singles.tile([128, D], F32)
            b_rep = singles.tile([128, D], F32)
            nc.gpsimd.partition_broadcast(g_rep[:], g_row[:])
            nc.gpsimd.partition_broadcast(b_rep[:], b_row[:])

            # ---- conv outputs (kept in SBUF, single wide tiles) ----
            # yqt/ykt: [dd, l] transposed conv out: [128, (t, l)] = [128, 8*512]
            yqt = ypool.tile([128, 8, LC], F32R, name="yqt")
            ykt = ypool.tile([128, 8, LC], F32R, name="ykt")
            # yv: [l, dd] conv out: [128, (t, dd)] = [128, 4*1024]
            yv = ypool.tile([128, 4, D], BF16, name="yv")
            # ctx_hd: [dd, l] context: [128, (c, l)] = [128, 8*512]
            ctx_hd = ypool.tile([128, 8, LC], BF16, name="ctx_hd")

            # ================= Phase C: the three convolutions =================
            with (
                tc.tile_pool(name="xt", bufs=1) as xtp,
                tc.tile_pool(name="wstream", bufs=3) as wsp,
                tc.tile_pool(name="ps_conv", bufs=1, space="PSUM") as psc,
            ):
                xq_sb = [xtp.tile([128, LC + 2], F32R, tag=f"xq{i}", name=f"xq{i}")
                         for i in range(8)]
                xk_sb = [xtp.tile([128, LC + 2], F32R, tag=f"xk{i}", name=f"xk{i}")
                         for i in range(8)]
                xv_sb = [xtp.tile([128, LC + 2], BF16, tag=f"xv{i}", name=f"xv{i}")
                         for i in range(8)]
                for i in range(8):
                    sl = slice(128 * i, 128 * i + 128)
                    nc.sync.dma_start(xq_sb[i], xq[sl, :])
                    nc.sync.dma_start(xk_sb[i], xk[sl, :])
                    nc.sync.dma_start(xv_sb[i], xv[sl, :])

                # q/k convs: out[dd_tile, l] += W_k[i, dd].T @ X[i, l+k]
                for name, wdram, ysb, xsb, bias_sb in (
                    ("q", wq, yqt, xq_sb, bq_sb), ("k", wk, ykt, xk_sb, bk_sb),
                ):
                    psum = [psc.tile([128, LC], F32, tag=f"pc{ot}", name=f"pc{ot}")
                            for ot in range(8)]
                    for step, (k3, ic) in enumerate(
                            [(a, b_) for a in range(KW) for b_ in range(8)]):
                        wslab = wsp.tile([128, D], F32R, tag="wslab")
                        nc.sync.dma_start(
                            wslab, wdram[k3, 128 * ic:128 * ic + 128, :])
                        for ot in range(8):
                            nc.tensor.matmul(
                                psum[ot][:],
                                wslab[:, 128 * ot:128 * ot + 128],
                                xsb[ic][:, k3:k3 + LC],
                                start=(step == 0), stop=(step == KW * 8 - 1),
                            )
                    for ot in range(8):
                        nc.scalar.activation(out=ysb[:, ot, :], in_=psum[ot][:],
                                             func=AF.Gelu,
                                             bias=bias_sb[:, ot:ot + 1])

                # v conv: out[l_tile, dd] += X[i, l+k].T @ W_k[i, dd]
                psum = [psc.tile([128, LC], F32, tag=f"pc{ot}", name=f"pc{ot}")
                        for ot in range(8)]
                for step, (k3, ic) in enumerate(
                        [(a, b_) for a in range(KW) for b_ in range(8)]):
                    wslab = wsp.tile([128, D], BF16, tag="wslab")
                    nc.sync.dma_start(
                        wslab, wv[k3, 128 * ic:128 * ic + 128, :])
                    for lt in range(4):
                        for nh in range(2):
                            nc.tensor.matmul(
                                psum[lt * 2 + nh][:],
                                xv_sb[ic][:, k3 + 128 * lt:k3 + 128 * lt + 128],
                                wslab[:, 512 * nh:512 * nh + 512],
                                start=(step == 0), stop=False,
                            )
                for lt in range(4):
                    for nh in range(2):
                        nc.tensor.matmul(
                            psum[lt * 2 + nh][:], ones_row[0:1, :],
                            bv_sb[0:1, 512 * nh:512 * nh + 512],
                            start=False, stop=True)
                for lt in range(4):
                    for nh in range(2):
                        nc.scalar.activation(
                            out=yv[:, lt, 512 * nh:512 * nh + 512],
                            in_=psum[lt * 2 + nh][:], func=AF.Gelu)

            # ================= Phase A: attention per group =================
            KP = 64  # contraction rows per S matmul (row-group packed x2)
            with (
                tc.tile_pool(name="qk", bufs=2) as qkp,
                tc.tile_pool(name="vv", bufs=2) as vvp,
                tc.tile_pool(name="cc", bufs=3) as ccp,
                tc.tile_pool(name="aa", bufs=2) as aap,
                tc.tile_pool(name="lc", bufs=10) as lcp,
                tc.tile_pool(name="sm", bufs=2) as smp,
                tc.tile_pool(name="ps_s", bufs=2, space="PSUM") as pss,
                tc.tile_pool(name="ps_x", bufs=1, space="PSUM") as psx,
            ):
                for g in range(NG):
                    l0 = 64 * g
                    # -- gather Q'^T, K'^T [j, r'] (r' = h*64+l), fp32r --
                    qg = qkp.tile([128, 1024], F32R, tag="qg")
                    kg = qkp.tile([128, 1024], F32R, tag="kg")
                    qr = qkp.tile([128, 1024], F32R, tag="qr", bufs=3)
                    # h-even from source rows 0:64, h-odd from rows 64:128;
                    # dest cols r' = h*64 + l with h = 2t (+1)
                    for par in range(2):
                        for dup in range(2):
                            qv = qg[64 * dup:64 * dup + 64, :].rearrange(
                                "p (t l) -> p t l", t=16)
                            kv = kg[64 * dup:64 * dup + 64, :].rearrange(
                                "p (t l) -> p t l", t=16)
                            nc.sync.dma_start(
                                qv[:, par::2, :],
                                yqt[64 * par:64 * par + 64, :, l0:l0 + 64])
                            nc.sync.dma_start(
                                kv[:, par::2, :],
                                ykt[64 * par:64 * par + 64, :, l0:l0 + 64])
                    # Q in r-order (r = l*16+h) via shuffled-write DVE copy
                    nc.vector.tensor_copy(
                        qr[:, :].rearrange("p (l h) -> p h l", h=16),
                        qg[:, :])
                    kr = qkp.tile([128, 1024], F32R, tag="kr", bufs=3)
                    nc.vector.tensor_copy(
                        kr[:, :].rearrange("p (l h) -> p h l", h=16),
                        kg[:, :])
                    # bf16 copies for the S^T pass (feeds bf16 ctx path only)
                    qb = qkp.tile([128, 1024], BF16, tag="qb", bufs=3)
                    kb = qkp.tile([128, 1024], BF16, tag="kb", bufs=3)
                    nc.vector.tensor_copy(qb[:, :], qg[:, :].bitcast(F32))
                    nc.vector.tensor_copy(kb[:, :], kg[:, :].bitcast(F32))

                    # -- gather V' [r', j] bf16: [128, (c, j)] = [128, 512] --
                    vg = vvp.tile([128, 8, HD], BF16, tag="vg", name="vg")
                    yv_g = yv[:, g // 2, :].rearrange("p (h j) -> p h j", h=16)
                    for par in range(2):
                        nc.sync.dma_start(
                            vg[64 * par:64 * par + 64, :, :],
                            yv_g[(g % 2) * 64:(g % 2) * 64 + 64, par::2, :])

                    # -- Vcolsum [64, 1] (negated, for the rank-1 fold) --
                    vcs_ps = psx.tile([64, 1], F32, tag="vcs")
                    for c in range(8):
                        nc.tensor.matmul(vcs_ps[:], vg[:, c, :], ones_bf[:],
                                         start=(c == 0), stop=(c == 7))
                    vcs = smp.tile([64, 1], F32, tag="vcs_sb")
                    nc.vector.tensor_scalar_mul(vcs[:], vcs_ps[:], -1.0)

                    # -- PASS 1: S rows in r-order; attn out + row sums --
                    s_cols = smp.tile([128, 8], F32, tag="s_cols", bufs=3)
                    for t in range(8):
                        ps = pss.tile([128, 1024], F32, tag="s")
                        for nh in range(2):
                            rg = slice(64 * nh, 64 * nh + 64)
                            nc.tensor.matmul(
                                ps[:, 512 * nh:512 * nh + 512],
                                qr[rg, 128 * t:128 * t + 128],
                                kr[rg, 512 * nh:512 * nh + 512],
                                start=True, stop=True)
                        c_sb = ccp.tile([128, 1024], F32, tag="c")
                        # clamp + rowsum accum (cols already r-ordered)
                        nc.vector.tensor_scalar(
                            out=c_sb[:], in0=ps[:], scalar1=0.2, scalar2=0.0,
                            op0=OP.max, op1=OP.add,
                            accum_out=s_cols[:, t:t + 1])
                        inv_s = smp.tile([128, 1], F32, tag="inv_s")
                        nc.vector.reciprocal(inv_s[:], s_cols[:, t:t + 1])
                        a_sb = aap.tile([128, 1024], F32, tag="a")
                        nc.scalar.activation(out=a_sb[:], in_=c_sb[:],
                                             func=AF.Ln, scale=inv_s[:])
                        nc.sync.dma_start(
                            attn_out[g, 128 * t:128 * t + 128, :], a_sb[:])

                    # -- PASS 2: S^T in r'-order -> log(max(S^T, 0.2)) bf16 --
                    logct = [lcp.tile([128, 1024], BF16, tag="logct", name="logct")
                             for _ in range(8)]
                    for t in range(8):
                        ps = pss.tile([128, 1024], F32, tag="s")
                        for nh in range(2):
                            rg = slice(64 * nh, 64 * nh + 64)
                            nc.tensor.matmul(
                                ps[:, 512 * nh:512 * nh + 512],
                                kb[rg, 128 * t:128 * t + 128],
                                qb[rg, 512 * nh:512 * nh + 512],
                                start=True, stop=True)
                        ct = ccp.tile([128, 1024], F32, tag="ct")
                        nc.vector.tensor_scalar_max(ct[:], ps[:], 0.2)
                        nc.scalar.activation(out=logct[t][:], in_=ct[:],
                                             func=AF.Ln)

                    # -- log(s') replicated row in r'-order [64, 1024] --
                    logs_c = smp.tile([128, 8], F32, tag="logs_c")
                    nc.scalar.activation(out=logs_c[:], in_=s_cols[:],
                                         func=AF.Ln)
                    lt_ps = psx.tile([8, 128], F32, tag="lt")
                    nc.tensor.transpose(lt_ps[:], logs_c[:], identity_f[:])
                    lt_sb = smp.tile([8, 128], F32, tag="lt_sb")
                    nc.vector.tensor_copy(lt_sb[:], lt_ps[:])
                    row_r = smp.tile([1, 1024], F32, tag="row_r")
                    nc.sync.dma_start(
                        row_r[0:1, :].rearrange("o (t p) -> o t p", t=8),
                        lt_sb[:, :])
                    row_rp = smp.tile([1, 1024], F32, tag="row_rp")
                    # r -> r' shuffle: out[h*64+l] = in[l*16+h]
                    nc.vector.tensor_copy(
                        row_rp[0:1, :].rearrange("o (h l) -> o l h", h=16),
                        row_r[0:1, :])
                    logs_rep = smp.tile([64, 1024], F32, tag="logs_rep")
                    nc.gpsimd.partition_broadcast(logs_rep[:], row_rp[:])

                    # -- PASS 3: context^T = V'^T @ logct - Vcs x log(s') --
                    ctxT = aap.tile([64, 1024], BF16, tag="ctxT")
                    for nh in range(2):
                        px = psx.tile([64, 512], F32, tag="x")
                        for c in range(8):
                            nc.tensor.matmul(
                                px[:], vg[:, c, :],
                                logct[c][:, 512 * nh:512 * nh + 512],
                                start=(c == 0), stop=(c == 7))
                        nc.vector.scalar_tensor_tensor(
                            out=ctxT[:, 512 * nh:512 * nh + 512],
                            in0=logs_rep[:, 512 * nh:512 * nh + 512],
                            scalar=vcs[:], in1=px[:],
                            op0=OP.mult, op1=OP.add)

                    # -- scatter context into ctx_hd [dd, l]: 2 DMAs --
                    ctxT_v = ctxT[:, :].rearrange("p (h l) -> p h l", h=16)
                    for par in range(2):
                        nc.sync.dma_start(
                            ctx_hd[64 * par:64 * par + 64, :, l0:l0 + 64],
                            ctxT_v[:, par::2, :])

            # ================= Phase P: projection + layernorm =================
            with (
                tc.tile_pool(name="wf", bufs=1) as wfp,
                tc.tile_pool(name="pp", bufs=2) as ppp,
                tc.tile_pool(name="ps_p", bufs=2, space="PSUM") as psp,
            ):
                wf_sb = [wfp.tile([128, D], BF16, tag=f"wf{c}", name=f"wfsb{c}")
                         for c in range(8)]
                for c in range(8):
                    nc.sync.dma_start(wf_sb[c],
                                      wf[128 * c:128 * c + 128, :])
                for lt in range(4):
                    ps = psp.tile([128, 1024], F32, tag="p")
                    for nh in range(2):
                        for c in range(8):
                            nc.tensor.matmul(
                                ps[:, 512 * nh:512 * nh + 512],
                                ctx_hd[:, c, 128 * lt:128 * lt + 128],
                                wf_sb[c][:, 512 * nh:512 * nh + 512],
                                start=(c == 0), stop=(c == 7))
                    res_sb = ppp.tile([128, D], F32, tag="res")
                    nc.sync.dma_start(res_sb,
                                      res[128 * lt:128 * lt + 128, :])
                    x_sb = ppp.tile([128, D], F32, tag="x")
                    nc.vector.tensor_tensor(x_sb[:], ps[:], res_sb[:], OP.add)
                    # layernorm stats (1024 = 2 x 512 bn_stats subgroups)
                    stats = ppp.tile([128, 2, 6], F32, tag="stats")
                    xg = x_sb[:].rearrange("p (s d) -> p s d", s=2)
                    for sg in range(2):
                        nc.vector.bn_stats(out=stats[:, sg, :],
                                           in_=xg[:, sg, :])
                    mv = ppp.tile([128, 2], F32, tag="mv")
                    nc.vector.bn_aggr(out=mv[:], in_=stats[:])
                    std = ppp.tile([128, 1], F32, tag="std")
                    nc.scalar.activation(out=std[:], in_=mv[:, 1:2],
                                         func=AF.Sqrt, bias=eps_sb[:])
                    istd = ppp.tile([128, 1], F32, tag="istd")
                    nc.vector.reciprocal(istd[:], std[:])
                    y_sb = ppp.tile([128, D], F32, tag="y")
                    nc.vector.tensor_scalar(
                        out=y_sb[:], in0=x_sb[:],
                        scalar1=mv[:, 0:1], scalar2=istd[:],
                        op0=OP.subtract, op1=OP.mult)
                    z_sb = ppp.tile([128, D], F32, tag="z")
                    nc.vector.tensor_tensor(z_sb[:], y_sb[:], g_rep[:],
                                            OP.mult)
                    o_sb = ppp.tile([128, D], F32, tag="o")
                    nc.vector.tensor_tensor(o_sb[:], z_sb[:], b_rep[:],
                                            OP.add)
                    nc.sync.dma_start(out[128 * lt:128 * lt + 128, :],
                                      o_sb[:])

    nc.compile()
    return nc


_CACHED_NC = None
LAST_RESULTS = None


def _get_nc():
    global _CACHED_NC
    if _CACHED_NC is None:
        _CACHED_NC = build_core_program()
    return _CACHED_NC


def kernel(key, value, query, wk, bk, wv, bv, wq, bq, wf, bf, ln_g, ln_b):
    key = np.asarray(key, np.float32)
    value = np.asarray(value, np.float32)
    query = np.asarray(query, np.float32)
    f32 = lambda x: np.asarray(x, np.float32)
    wk, bk, wv, bv, wq, bq = map(f32, (wk, bk, wv, bv, wq, bq))
    wf, bf, ln_g, ln_b = map(f32, (wf, bf, ln_g, ln_b))

    # Weights pre-transposed host-side to [k, i, dd] ([dd, o] for wf). Conv
    # biases ride along: bq/bk as per-partition gelu bias columns, bv as a
    # rank-1 matmul row, bf folded into the residual.
    wq_t = np.ascontiguousarray(wq.transpose(2, 1, 0))
    wk_t = np.ascontiguousarray(wk.transpose(2, 1, 0))
    wv_t = np.ascontiguousarray(wv.transpose(2, 1, 0)).astype(ml_dtypes.bfloat16)
    wf_t = np.ascontiguousarray(wf.T).astype(ml_dtypes.bfloat16)

    nc = _get_nc()

    in_maps = []
    for core in range(8):
        b, half = core // 2, core % 2
        s0 = half * LC
        cols = (np.arange(s0 - 2, s0 + LC) % L)
        in_maps.append({
            "xq": np.ascontiguousarray(query[b].T[:, cols]),
            "xk": np.ascontiguousarray(key[b].T[:, cols]),
            "xv": np.ascontiguousarray(value[b].T[:, cols]).astype(
                ml_dtypes.bfloat16),
            "wq": wq_t, "wk": wk_t, "wv": wv_t, "wf": wf_t,
            "bqc": np.ascontiguousarray(bq.reshape(8, 128).T),
            "bkc": np.ascontiguousarray(bk.reshape(8, 128).T),
            "bvr": np.ascontiguousarray(bv[None, :]).astype(ml_dtypes.bfloat16),
            "res": np.ascontiguousarray(query[b, s0:s0 + LC] + bf),
            "lng": np.ascontiguousarray(ln_g[None, :]),
            "lnb": np.ascontiguousarray(ln_b[None, :]),
        })

    import os
    trace = bool(int(os.environ.get("KERNEL_TRACE", "0")))
    results = run_bass_kernel_spmd(nc, in_maps, core_ids=list(range(8)),
                                   trace=trace)
    global LAST_RESULTS
    LAST_RESULTS = results

    output = np.zeros((B, L, D), np.float32)
    attn = np.zeros((B * H, L, L), np.float32)
    for core in range(8):
        b, half = core // 2, core % 2
        s0 = half * LC
        r = results.results[core]
        output[b, s0:s0 + LC] = r["out"]
        attn[b * H + half * NG:b * H + half * NG + NG] = r["attn_out"]
    return (output, attn)


if __name__ == "__main__":
    # quick self-build check
    nc = build_core_program()
    print("build + compile OK")
